# revision 8
# baseline (speedup 1.0000x reference)
"""Hyperbolic (Poincare-ball) average pooling 1D — Trainium2 Bass kernel (v2).

Problem: x (16, 256, 16384) f32, kernel=stride=4, manifold dim = channels (256).
Math (c=1), per window position:
    n2   = sum_C x^2                     (per input position)
    r    = 1/(1-n2)
    num  = sum_j r_j x_j  (window of 4)  ; den = sum_j r_j ; D = den - 2
    out  = num * g,  g = 1/(D + sqrt(D^2 - s)),  s = sum_C num^2

v2 strategy (all bf16 on the wire, f32 accumulation):
 - shard over batch (2 rows/core); host pre-transposes to (positions,
   channels+ones+pad) bf16 tiles [32, 128, 8*258].
 - n2: ScalarE/DVE square (bf16), then a PE identity-matmul 4:1 fold into
   PSUM ([128,8,64]) and a short DVE tensor_reduce — cuts the 1x reduce cost.
 - r = 1/(1-n2) via DVE iterative reciprocal (exact); no ACT ln/exp tables.
 - window sum as PE matmuls with 32-column stationary (mask32 * r) via
   col-tiling (tile_position), 258-wide moving incl. ones column for den.
 - evac num||den to bf16 SBUF (ScalarE), s via DVE TT-square (2x) + reduce,
   g chain on [128,16] batches: ScalarE Square/Sqrt + DVE reciprocal.
 - out = num * g via DVE tensor_scalar (2x), one 1MB DMA per 8 tiles.
"""

import sys

sys.path.insert(0, "/opt/trn_rl_repo")

import copy
import numpy as np
from ml_dtypes import bfloat16

import concourse.bass as bass
import concourse.mybir as mybir
from concourse import tile
from concourse.bass_utils import run_bass_kernel_spmd
from contextlib import ExitStack

F32 = mybir.dt.float32
BF16 = mybir.dt.bfloat16
AF = mybir.ActivationFunctionType
ALU = mybir.AluOpType

B, C, L = 16, 256, 16384
KERN = 4
T = L // KERN             # 4096 out positions per batch row
N_CORES = 8
B_PER = B // N_CORES      # 2
POS = B_PER * L           # 32768 input positions per core
OPOS = POS // KERN        # 8192 out positions per core
CPC = 258                 # 256 channels + ones col + zero pad
Q = 8                     # q-slots per x-tile
TILE_POS = 128 * Q        # 1024 input positions per x-tile
N_TILES = POS // TILE_POS  # 32
RG = 4                    # tiles per r-chain batch
GRP = 8                   # tiles per output group (g-chain, out DMA)
NG = N_TILES // GRP       # 4

# --- tuning knobs ---
N_SQ_DVE = 8      # how many tiles (of 32) square on DVE instead of ScalarE
WR_POOL = True    # build wr32 on GpSimd(Pool) instead of DVE
TTSQ_POOL = True  # square numsb on Pool instead of DVE


def _split_multi_waits(nc, max_waits=1):
    """walrus in this container rejects >1 sync-wait on one instruction;
    split extras into preceding single-wait NOPs on the same engine."""
    n_new = 0
    for bb in nc.m.functions[0].blocks:
        new_list = []
        for inst in bb.instructions:
            si = getattr(inst, "sync_info", None)
            if si is not None and si.on_wait and len(si.on_wait) > max_waits:
                extra = si.on_wait[:-max_waits]
                si_keep = si.on_wait[-max_waits:]
                for w in extra:
                    nop = mybir.InstNoOp(
                        name=f"{inst.name}-wsplit{n_new}", ins=[], outs=[]
                    )
                    nop.engine = inst.engine
                    nsi = copy.deepcopy(si)
                    nsi.on_wait = [w]
                    nsi.on_update = []
                    nop.sync_info = nsi
                    new_list.append(nop)
                    n_new += 1
                si.on_wait = si_keep
            new_list.append(inst)
        bb.instructions = new_list
    return n_new


def _register_const_ap(nc, value):
    t = nc.alloc_sbuf_tensor(f"const-float32-{value}", [128, 1], F32)
    nc.gpsimd.memset(t.ap(), value)
    nc.const_aps.aps[(F32, value)] = t.ap()


def build_nc(split_waits=True):
    nc = bass.Bass()
    _register_const_ap(nc, -2.0)
    nc.all_engine_barrier()
    xt = nc.declare_dram_parameter("xt", [N_TILES, 128, Q * CPC], BF16, isOutput=False)
    mask32 = nc.declare_dram_parameter("mask32", [128, 32], BF16, isOutput=False)
    mask8 = nc.declare_dram_parameter("mask8", [128, Q * 32], BF16, isOutput=False)
    ident = nc.declare_dram_parameter("ident", [128, 128], BF16, isOutput=False)
    out = nc.declare_dram_parameter("out", [NG, 128, GRP * 2 * 256], BF16, isOutput=True)

    NRG = N_TILES // RG  # 8 r-groups

    with tile.TileContext(nc) as tc:
        with ExitStack() as ctx:
            xpool = ctx.enter_context(tc.tile_pool(name="x", bufs=3 * RG))
            sqpool = ctx.enter_context(tc.tile_pool(name="sq", bufs=6))
            wrpool = ctx.enter_context(tc.tile_pool(name="wr", bufs=4))
            nbpool = ctx.enter_context(tc.tile_pool(name="nb", bufs=2))
            snpool = ctx.enter_context(tc.tile_pool(name="sn", bufs=3))
            stpool = ctx.enter_context(tc.tile_pool(name="st", bufs=3))
            opool = ctx.enter_context(tc.tile_pool(name="o", bufs=2))
            cpool = ctx.enter_context(tc.tile_pool(name="c", bufs=1))
            mpspool = ctx.enter_context(tc.tile_pool(name="mps", bufs=4, space="PSUM"))
            fpspool = ctx.enter_context(tc.tile_pool(name="fps", bufs=3, space="PSUM"))

            mask_t = cpool.tile([128, 32], BF16, tag="mask")
            nc.sync.dma_start(mask_t[:], mask32[:, :])
            mask8_t = cpool.tile([128, Q, 32], BF16, tag="mask8")
            nc.sync.dma_start(mask8_t[:], mask8.rearrange("p (q t) -> p q t", q=Q))
            id_t = cpool.tile([128, 128], BF16, tag="id")
            nc.sync.dma_start(id_t[:], ident[:, :])

            # group state (created lazily per group)
            numsb_g = {}
            s_gs = {}
            o_ts = {}

            def group_state(grp):
                if grp not in numsb_g:
                    numsb_g[grp] = nbpool.tile([128, GRP, 2, CPC], BF16, tag="numsb", name=f"numsb{grp}")
                    s_gs[grp] = stpool.tile([128, 2 * GRP], F32, tag="s", name=f"sg{grp}")
                    o_ts[grp] = opool.tile([128, GRP, 2, 256], BF16, tag="o", name=f"og{grp}")
                return numsb_g[grp], s_gs[grp], o_ts[grp]

            a_state = {}   # tile i -> x_t
            m_state = {}   # rg -> m_g
            r_state = {}   # rg -> r_g

            def dma_tile(i):
                x_t = xpool.tile([128, Q, CPC], BF16, tag="x", name=f"x{i}")
                nc.sync.dma_start(x_t[:], xt[i].rearrange("p (q c) -> p q c", q=Q))
                a_state[i] = x_t

            def a_tile(i):
                """square + PE-fold + n2 reduce for one tile."""
                rg = i // RG
                if i % RG == 0:
                    m_state[rg] = stpool.tile(
                        [128, Q * RG], F32, tag="m", name=f"m{rg}"
                    )
                m_g = m_state[rg]
                j = i % RG
                x_t = a_state[i]
                sq_t = sqpool.tile([128, Q, 256], BF16, tag="sq")
                if N_SQ_DVE > 0 and i % (N_TILES // max(N_SQ_DVE, 1)) == 0:
                    nc.vector.tensor_tensor(
                        out=sq_t[:],
                        in0=x_t[:, :, 0:256],
                        in1=x_t[:, :, 0:256],
                        op=ALU.mult,
                    )
                else:
                    nc.scalar.activation(sq_t[:], x_t[:, :, 0:256], AF.Square)
                fps = fpspool.tile([128, Q, 64], F32, tag="fps")
                for ck in range(4):
                    nc.tensor.matmul(
                        fps[:],
                        id_t[:],
                        sq_t[:, :, 64 * ck : 64 * (ck + 1)],
                        start=(ck == 0),
                        stop=(ck == 3),
                    )
                nc.vector.tensor_reduce(
                    m_g[:, j * Q : (j + 1) * Q],
                    fps[:],
                    axis=mybir.AxisListType.X,
                    op=ALU.add,
                )

            def r_chain(rg):
                """r = 1/(1 - n2) for one batch of RG tiles."""
                m_g = m_state.pop(rg)
                w1 = stpool.tile([128, Q * RG], F32, tag="w1")
                nc.vector.tensor_scalar(
                    out=w1[:], in0=m_g[:], scalar1=-1.0, scalar2=1.0,
                    op0=ALU.mult, op1=ALU.add,
                )
                r_g = stpool.tile([128, Q * RG], F32, tag="r")
                nc.vector.reciprocal(r_g[:], w1[:])
                r_state[rg] = r_g

            def b_tile(i):
                """wr, window matmuls, evac, s for one tile."""
                grp = i // GRP
                numsb, s_g, o_t = group_state(grp)
                x_t = a_state.pop(i)
                rg, j = i // RG, i % RG
                r_g = r_state[rg]
                jj = i % GRP
                wr_t = wrpool.tile([128, Q, 32], BF16, tag="wr")
                r_b = (
                    r_g[:, j * Q : (j + 1) * Q]
                    .rearrange("p (q o) -> p q o", o=1)
                    .broadcast_to([128, Q, 32])
                )
                m_b = mask8_t[:]
                eng = nc.gpsimd if WR_POOL else nc.vector
                eng.tensor_tensor(out=wr_t[:], in0=m_b, in1=r_b, op=ALU.mult)

                pss = []
                for bk in range(2):
                    ps = mpspool.tile([128, CPC], F32, tag="ps")
                    for ql in range(4):
                        q = 4 * bk + ql
                        nc.tensor.matmul(
                            ps[32 * ql : 32 * (ql + 1), :],
                            wr_t[:, q],
                            x_t[:, q, :],
                            start=True,
                            stop=True,
                            tile_position=(0, 32 * ql),
                        )
                    pss.append(ps)
                for bk in range(2):
                    nc.scalar.copy(numsb[:, jj, bk], pss[bk][:])
                sqn = snpool.tile([128, 2, 256], BF16, tag="sqn")
                teng = nc.gpsimd if TTSQ_POOL else nc.vector
                teng.tensor_tensor(
                    out=sqn[:],
                    in0=numsb[:, jj, :, 0:256],
                    in1=numsb[:, jj, :, 0:256],
                    op=ALU.mult,
                )
                nc.vector.tensor_reduce(
                    s_g[:, 2 * jj : 2 * jj + 2],
                    sqn[:],
                    axis=mybir.AxisListType.X,
                    op=ALU.add,
                )

            def phase_tail(grp):
                """den, g chain, out scale, store for one group of GRP tiles."""
                numsb = numsb_g.pop(grp)
                s_g = s_gs.pop(grp)
                o_t = o_ts.pop(grp)
                den_s = stpool.tile([128, 2 * GRP], F32, tag="den")
                nc.vector.tensor_copy(den_s[:], numsb[:, :, :, 256])
                d2 = stpool.tile([128, 2 * GRP], F32, tag="d2")
                nc.scalar.activation(d2[:], den_s[:], AF.Square, bias=-2.0)
                q2 = stpool.tile([128, 2 * GRP], F32, tag="q2")
                nc.vector.tensor_tensor(
                    out=q2[:], in0=d2[:], in1=s_g[:], op=ALU.subtract
                )
                u = stpool.tile([128, 2 * GRP], F32, tag="u")
                nc.scalar.activation(u[:], q2[:], AF.Sqrt)
                du = stpool.tile([128, 2 * GRP], F32, tag="du")
                nc.vector.scalar_tensor_tensor(
                    out=du[:], in0=den_s[:], scalar=-2.0, in1=u[:],
                    op0=ALU.add, op1=ALU.add,
                )
                g_s = stpool.tile([128, 2 * GRP], F32, tag="g")
                nc.vector.reciprocal(g_s[:], du[:])
                for jj in range(GRP):
                    for bk in range(2):
                        nc.vector.tensor_scalar(
                            out=o_t[:, jj, bk],
                            in0=numsb[:, jj, bk, 0:256],
                            scalar1=g_s[:, 2 * jj + bk : 2 * jj + bk + 1],
                            scalar2=None,
                            op0=ALU.mult,
                        )
                nc.sync.dma_start(
                    out[grp], o_t[:].rearrange("p a b c -> p (a b c)")
                )

            # software pipeline, tile-granular: DMA leads A by LEAD tiles,
            # A leads B by RG tiles; backward-dep work (B) emitted first.
            LEAD = 6
            for i in range(min(LEAD, N_TILES)):
                dma_tile(i)
            for k in range(N_TILES + RG):
                if k + LEAD < N_TILES:
                    dma_tile(k + LEAD)
                if k >= RG:
                    i = k - RG
                    b_tile(i)
                    if i % GRP == GRP - 1:
                        phase_tail(i // GRP)
                if k < N_TILES:
                    a_tile(k)
                    if k % RG == RG - 1:
                        r_chain(k // RG)

    if split_waits:
        _split_multi_waits(nc)
    return nc


_NC_CACHE = None


def _get_nc():
    global _NC_CACHE
    if _NC_CACHE is None:
        _NC_CACHE = build_nc()
    return _NC_CACHE


def _make_mask32():
    m = np.zeros((128, 32), dtype=np.float32)
    p = np.arange(128)
    m[p, p // 4] = 1.0
    return m.astype(bfloat16)


def prepare_core_inputs(x):
    """x: (16, 256, 16384) f32 -> list of per-core input dicts (bf16)."""
    mask32 = _make_mask32()
    mask8 = np.ascontiguousarray(np.tile(mask32.astype(np.float32), (1, Q))).astype(bfloat16)
    ident = np.eye(128, dtype=np.float32).astype(bfloat16)
    in_maps = []
    for k in range(N_CORES):
        xs = x[k * B_PER : (k + 1) * B_PER]  # (2, 256, L)
        xtf = np.empty((POS, CPC), dtype=np.float32)
        xtf[:, :C] = xs.transpose(0, 2, 1).reshape(POS, C)
        xtf[:, C] = 1.0
        xtf[:, C + 1] = 0.0
        # partition-major per-tile layout: (tile, p, q*CPC)
        xtb = np.ascontiguousarray(
            xtf.reshape(N_TILES, Q, 128, CPC).transpose(0, 2, 1, 3)
        ).reshape(N_TILES, 128, Q * CPC).astype(bfloat16)
        in_maps.append({"xt": xtb, "mask32": mask32, "mask8": mask8, "ident": ident})
    return in_maps


def assemble_output(results):
    outs = []
    for k in range(N_CORES):
        o = results[k]["out"]  # (NG, 128, GRP*2*256) bf16
        o = np.asarray(o).astype(np.float32)
        # [NG, p=128, jj, bk, c] -> out position = ((g*GRP+jj)*2 + bk)*128 + p
        o = o.reshape(NG, 128, GRP, 2, 256).transpose(0, 2, 3, 1, 4).reshape(OPOS, 256)
        outs.append(o.reshape(B_PER, T, C).transpose(0, 2, 1))
    return np.ascontiguousarray(np.concatenate(outs, axis=0))


def kernel(x):
    x = np.ascontiguousarray(x, dtype=np.float32)
    nc = _get_nc()
    in_maps = prepare_core_inputs(x)
    res = run_bass_kernel_spmd(nc, in_maps, core_ids=list(range(N_CORES)))
    return assemble_output(res.results)


# revision 13
# speedup vs baseline: 1.0604x; 1.0604x over previous
"""Hyperbolic (Poincare-ball) average pooling 1D — Trainium2 Bass kernel (v2).

Problem: x (16, 256, 16384) f32, kernel=stride=4, manifold dim = channels (256).
Math (c=1), per window position:
    n2   = sum_C x^2                     (per input position)
    r    = 1/(1-n2)
    num  = sum_j r_j x_j  (window of 4)  ; den = sum_j r_j ; D = den - 2
    out  = num * g,  g = 1/(D + sqrt(D^2 - s)),  s = sum_C num^2

v2 strategy (all bf16 on the wire, f32 accumulation):
 - shard over batch (2 rows/core); host pre-transposes to (positions,
   channels+ones+pad) bf16 tiles [32, 128, 8*258].
 - n2: ScalarE/DVE square (bf16), then a PE identity-matmul 4:1 fold into
   PSUM ([128,8,64]) and a short DVE tensor_reduce — cuts the 1x reduce cost.
 - r = 1/(1-n2) via DVE iterative reciprocal (exact); no ACT ln/exp tables.
 - window sum as PE matmuls with 32-column stationary (mask32 * r) via
   col-tiling (tile_position), 258-wide moving incl. ones column for den.
 - evac num||den to bf16 SBUF (ScalarE), s via DVE TT-square (2x) + reduce,
   g chain on [128,16] batches: ScalarE Square/Sqrt + DVE reciprocal.
 - out = num * g via DVE tensor_scalar (2x), one 1MB DMA per 8 tiles.
"""

import sys

sys.path.insert(0, "/opt/trn_rl_repo")

import copy
import numpy as np
from ml_dtypes import bfloat16

import concourse.bass as bass
import concourse.mybir as mybir
from concourse import tile
from concourse.bass_utils import run_bass_kernel_spmd
from contextlib import ExitStack

F32 = mybir.dt.float32
BF16 = mybir.dt.bfloat16
AF = mybir.ActivationFunctionType
ALU = mybir.AluOpType

B, C, L = 16, 256, 16384
KERN = 4
T = L // KERN             # 4096 out positions per batch row
N_CORES = 8
B_PER = B // N_CORES      # 2
POS = B_PER * L           # 32768 input positions per core
OPOS = POS // KERN        # 8192 out positions per core
CPC = 258                 # 256 channels + ones col + zero pad
Q = 8                     # q-slots per x-tile
TILE_POS = 128 * Q        # 1024 input positions per x-tile
N_TILES = POS // TILE_POS  # 32
RG = 4                    # tiles per r-chain batch
GRP = 8                   # tiles per output group (g-chain, out DMA)
NG = N_TILES // GRP       # 4

# --- tuning knobs ---
N_SQ_DVE = 6      # how many tiles (of 32) square on DVE instead of ScalarE
WR_POOL = True    # build wr32 on GpSimd(Pool) instead of DVE
TTSQ_POOL = True  # square numsb on Pool instead of DVE


def _split_multi_waits(nc, max_waits=1):
    """walrus in this container rejects >1 sync-wait on one instruction;
    split extras into preceding single-wait NOPs on the same engine."""
    n_new = 0
    for bb in nc.m.functions[0].blocks:
        new_list = []
        for inst in bb.instructions:
            si = getattr(inst, "sync_info", None)
            if si is not None and si.on_wait and len(si.on_wait) > max_waits:
                extra = si.on_wait[:-max_waits]
                si_keep = si.on_wait[-max_waits:]
                for w in extra:
                    nop = mybir.InstNoOp(
                        name=f"{inst.name}-wsplit{n_new}", ins=[], outs=[]
                    )
                    nop.engine = inst.engine
                    nsi = copy.deepcopy(si)
                    nsi.on_wait = [w]
                    nsi.on_update = []
                    nop.sync_info = nsi
                    new_list.append(nop)
                    n_new += 1
                si.on_wait = si_keep
            new_list.append(inst)
        bb.instructions = new_list
    return n_new


def _register_const_ap(nc, value):
    t = nc.alloc_sbuf_tensor(f"const-float32-{value}", [128, 1], F32)
    nc.gpsimd.memset(t.ap(), value)
    nc.const_aps.aps[(F32, value)] = t.ap()


def build_nc(split_waits=True):
    nc = bass.Bass()
    _register_const_ap(nc, -2.0)
    nc.all_engine_barrier()
    xt = nc.declare_dram_parameter("xt", [N_TILES, 128, Q * CPC], BF16, isOutput=False)
    mask32 = nc.declare_dram_parameter("mask32", [128, 32], BF16, isOutput=False)
    mask8 = nc.declare_dram_parameter("mask8", [128, Q * 32], BF16, isOutput=False)
    masktm = nc.declare_dram_parameter("masktm", [128, 32 * Q], BF16, isOutput=False)
    ident = nc.declare_dram_parameter("ident", [128, 128], BF16, isOutput=False)
    out = nc.declare_dram_parameter("out", [NG, 128, GRP * 2 * 256], BF16, isOutput=True)

    NRG = N_TILES // RG  # 8 r-groups

    with tile.TileContext(nc) as tc:
        with ExitStack() as ctx:
            xpool = ctx.enter_context(tc.tile_pool(name="x", bufs=19))
            sqpool = ctx.enter_context(tc.tile_pool(name="sq", bufs=6))
            wrpool = ctx.enter_context(tc.tile_pool(name="wr", bufs=4))
            nbpool = ctx.enter_context(tc.tile_pool(name="nb", bufs=3))
            snpool = ctx.enter_context(tc.tile_pool(name="sn", bufs=3))
            stpool = ctx.enter_context(tc.tile_pool(name="st", bufs=3))
            opool = ctx.enter_context(tc.tile_pool(name="o", bufs=2))
            cpool = ctx.enter_context(tc.tile_pool(name="c", bufs=1))
            mpspool = ctx.enter_context(tc.tile_pool(name="mps", bufs=4, space="PSUM"))
            fpspool = ctx.enter_context(tc.tile_pool(name="fps", bufs=3, space="PSUM"))

            mask_t = cpool.tile([128, 32], BF16, tag="mask")
            nc.sync.dma_start(mask_t[:], mask32[:, :])
            mask8_t = cpool.tile([128, Q, 32], BF16, tag="mask8")
            nc.sync.dma_start(mask8_t[:], mask8.rearrange("p (q t) -> p q t", q=Q))
            masktm_t = cpool.tile([128, 32, Q], BF16, tag="masktm")
            nc.sync.dma_start(masktm_t[:], masktm.rearrange("p (t q) -> p t q", t=32))
            id_t = cpool.tile([128, 128], BF16, tag="id")
            nc.sync.dma_start(id_t[:], ident[:, :])

            # group state (created lazily per group)
            numsb_g = {}
            s_gs = {}
            o_ts = {}

            def group_state(grp):
                if grp not in numsb_g:
                    numsb_g[grp] = nbpool.tile([128, GRP, 2, CPC], BF16, tag="numsb", name=f"numsb{grp}")
                    s_gs[grp] = stpool.tile([128, 2 * GRP], F32, tag="s", name=f"sg{grp}")
                    o_ts[grp] = opool.tile([128, GRP, 2, 256], BF16, tag="o", name=f"og{grp}")
                return numsb_g[grp], s_gs[grp], o_ts[grp]

            a_state = {}   # tile i -> x_t
            m_state = {}   # rg -> m_g
            r_state = {}   # rg -> r_g

            def dma_tile(i):
                x_t = xpool.tile([128, Q, CPC], BF16, tag="x", name=f"x{i}")
                nc.sync.dma_start(x_t[:], xt[i].rearrange("p (q c) -> p q c", q=Q))
                a_state[i] = x_t

            sq_state = {}

            def a1_tile(i):
                """square for one tile."""
                x_t = a_state[i]
                sq_t = sqpool.tile([128, Q, 256], BF16, tag="sq")
                if N_SQ_DVE > 0 and i % (N_TILES // max(N_SQ_DVE, 1)) == 0:
                    nc.vector.tensor_tensor(
                        out=sq_t[:],
                        in0=x_t[:, :, 0:256],
                        in1=x_t[:, :, 0:256],
                        op=ALU.mult,
                    )
                else:
                    nc.scalar.activation(sq_t[:], x_t[:, :, 0:256], AF.Square)
                sq_state[i] = sq_t

            def a2_tile(i):
                """PE-fold + n2 reduce for one tile."""
                rg = i // RG
                if i % RG == 0:
                    m_state[rg] = stpool.tile(
                        [128, Q * RG], F32, tag="m", name=f"m{rg}"
                    )
                m_g = m_state[rg]
                j = i % RG
                sq_t = sq_state.pop(i)
                fps = fpspool.tile([128, Q, 64], F32, tag="fps")
                for ck in range(4):
                    nc.tensor.matmul(
                        fps[:],
                        id_t[:],
                        sq_t[:, :, 64 * ck : 64 * (ck + 1)],
                        start=(ck == 0),
                        stop=(ck == 3),
                    )
                nc.vector.tensor_reduce(
                    m_g[:, j * Q : (j + 1) * Q],
                    fps[:],
                    axis=mybir.AxisListType.X,
                    op=ALU.add,
                )

            def r_chain(rg):
                """r = 1/(1 - n2) for one batch of RG tiles."""
                m_g = m_state.pop(rg)
                w1 = stpool.tile([128, Q * RG], F32, tag="w1")
                nc.vector.tensor_scalar(
                    out=w1[:], in0=m_g[:], scalar1=-1.0, scalar2=1.0,
                    op0=ALU.mult, op1=ALU.add,
                )
                r_g = stpool.tile([128, Q * RG], F32, tag="r")
                nc.vector.reciprocal(r_g[:], w1[:])
                r_state[rg] = r_g

            wr_state = {}

            def b0_tile(i):
                """wr stationary build for one tile."""
                rg, j = i // RG, i % RG
                r_g = r_state[rg]
                wr_t = wrpool.tile([128, Q, 32], BF16, tag="wr")
                r_b = (
                    r_g[:, j * Q : (j + 1) * Q]
                    .rearrange("p (q o) -> p q o", o=1)
                    .broadcast_to([128, Q, 32])
                )
                eng = nc.gpsimd if WR_POOL else nc.vector
                eng.tensor_tensor(out=wr_t[:], in0=mask8_t[:], in1=r_b, op=ALU.mult)
                wr_state[i] = wr_t

            def b1_tile(i):
                """window matmuls + evac for one tile."""
                grp = i // GRP
                numsb, s_g, o_t = group_state(grp)
                x_t = a_state.pop(i)
                wr_t = wr_state.pop(i)
                jj = i % GRP
                pss = []
                for bk in range(2):
                    ps = mpspool.tile([128, CPC], F32, tag="ps")
                    for ql in range(4):
                        q = 4 * bk + ql
                        nc.tensor.matmul(
                            ps[32 * ql : 32 * (ql + 1), :],
                            wr_t[:, q],
                            x_t[:, q, :],
                            start=True,
                            stop=True,
                            tile_position=(0, 32 * ql),
                        )
                    pss.append(ps)
                for bk in range(2):
                    nc.scalar.copy(numsb[:, jj, bk], pss[bk][:])

            def b2_tile(i):
                """s = sum num^2 for one tile."""
                grp = i // GRP
                numsb, s_g, o_t = group_state(grp)
                jj = i % GRP
                sqn = snpool.tile([128, 2, 256], BF16, tag="sqn")
                teng = nc.gpsimd if TTSQ_POOL else nc.vector
                teng.tensor_tensor(
                    out=sqn[:],
                    in0=numsb[:, jj, :, 0:256],
                    in1=numsb[:, jj, :, 0:256],
                    op=ALU.mult,
                )
                nc.vector.tensor_reduce(
                    s_g[:, 2 * jj : 2 * jj + 2],
                    sqn[:],
                    axis=mybir.AxisListType.X,
                    op=ALU.add,
                )

            def phase_tail(grp):
                """den, g chain, out scale, store for one group of GRP tiles."""
                numsb = numsb_g.pop(grp)
                s_g = s_gs.pop(grp)
                o_t = o_ts.pop(grp)
                den_s = stpool.tile([128, 2 * GRP], F32, tag="den")
                nc.vector.tensor_copy(den_s[:], numsb[:, :, :, 256])
                d2 = stpool.tile([128, 2 * GRP], F32, tag="d2")
                nc.scalar.activation(d2[:], den_s[:], AF.Square, bias=-2.0)
                q2 = stpool.tile([128, 2 * GRP], F32, tag="q2")
                nc.vector.tensor_tensor(
                    out=q2[:], in0=d2[:], in1=s_g[:], op=ALU.subtract
                )
                u = stpool.tile([128, 2 * GRP], F32, tag="u")
                nc.scalar.activation(u[:], q2[:], AF.Sqrt)
                du = stpool.tile([128, 2 * GRP], F32, tag="du")
                nc.vector.scalar_tensor_tensor(
                    out=du[:], in0=den_s[:], scalar=-2.0, in1=u[:],
                    op0=ALU.add, op1=ALU.add,
                )
                g_s = stpool.tile([128, 2 * GRP], F32, tag="g")
                nc.vector.reciprocal(g_s[:], du[:])
                for jj in range(GRP):
                    for bk in range(2):
                        nc.vector.tensor_scalar(
                            out=o_t[:, jj, bk],
                            in0=numsb[:, jj, bk, 0:256],
                            scalar1=g_s[:, 2 * jj + bk : 2 * jj + bk + 1],
                            scalar2=None,
                            op0=ALU.mult,
                        )
                nc.sync.dma_start(
                    out[grp], o_t[:].rearrange("p a b c -> p (a b c)")
                )

            # staged software pipeline (per-cycle emission order matters:
            # every engine's next instruction should have old, ready deps)
            LEAD = 10      # DMA leads a1
            LA2 = 2        # a2 (fold+TR) lags a1
            LB0 = 6        # b0 (wr) lag
            LB1 = 7        # b1 (MM+evac) lag
            LB2 = 9        # b2 (ttsq+sTR) lag
            TOT = LB2 + 1
            for i in range(min(LEAD, N_TILES)):
                dma_tile(i)
            for k in range(N_TILES + TOT):
                if k + LEAD < N_TILES:
                    dma_tile(k + LEAD)
                if k < N_TILES:
                    a1_tile(k)
                if RG <= k + RG - LB0 and k - LB0 < N_TILES and k >= LB0:
                    b0_tile(k - LB0)
                if k >= LB1 and k - LB1 < N_TILES:
                    b1_tile(k - LB1)
                if k >= LA2 and k - LA2 < N_TILES:
                    a2_tile(k - LA2)
                    if (k - LA2) % RG == RG - 1:
                        r_chain((k - LA2) // RG)
                if k >= LB2 and k - LB2 < N_TILES:
                    i2 = k - LB2
                    b2_tile(i2)
                    if i2 % GRP == GRP - 1:
                        phase_tail(i2 // GRP)

    if split_waits:
        _split_multi_waits(nc)
    return nc


_NC_CACHE = None


def _get_nc():
    global _NC_CACHE
    if _NC_CACHE is None:
        _NC_CACHE = build_nc()
    return _NC_CACHE


def _make_mask32():
    m = np.zeros((128, 32), dtype=np.float32)
    p = np.arange(128)
    m[p, p // 4] = 1.0
    return m.astype(bfloat16)


def prepare_core_inputs(x):
    """x: (16, 256, 16384) f32 -> list of per-core input dicts (bf16)."""
    mask32 = _make_mask32()
    mask8 = np.ascontiguousarray(np.tile(mask32.astype(np.float32), (1, Q))).astype(bfloat16)
    masktm = np.ascontiguousarray(
        np.repeat(mask32.astype(np.float32), Q, axis=1)
    ).astype(bfloat16)
    ident = np.eye(128, dtype=np.float32).astype(bfloat16)
    in_maps = []
    for k in range(N_CORES):
        xs = x[k * B_PER : (k + 1) * B_PER]  # (2, 256, L)
        xtf = np.empty((POS, CPC), dtype=np.float32)
        xtf[:, :C] = xs.transpose(0, 2, 1).reshape(POS, C)
        xtf[:, C] = 1.0
        xtf[:, C + 1] = 0.0
        # partition-major per-tile layout: (tile, p, q*CPC)
        xtb = np.ascontiguousarray(
            xtf.reshape(N_TILES, Q, 128, CPC).transpose(0, 2, 1, 3)
        ).reshape(N_TILES, 128, Q * CPC).astype(bfloat16)
        in_maps.append({"xt": xtb, "mask32": mask32, "mask8": mask8, "masktm": masktm, "ident": ident})
    return in_maps


def assemble_output(results):
    outs = []
    for k in range(N_CORES):
        o = results[k]["out"]  # (NG, 128, GRP*2*256) bf16
        o = np.asarray(o).astype(np.float32)
        # [NG, p=128, jj, bk, c] -> out position = ((g*GRP+jj)*2 + bk)*128 + p
        o = o.reshape(NG, 128, GRP, 2, 256).transpose(0, 2, 3, 1, 4).reshape(OPOS, 256)
        outs.append(o.reshape(B_PER, T, C).transpose(0, 2, 1))
    return np.ascontiguousarray(np.concatenate(outs, axis=0))


def kernel(x):
    x = np.ascontiguousarray(x, dtype=np.float32)
    nc = _get_nc()
    in_maps = prepare_core_inputs(x)
    res = run_bass_kernel_spmd(nc, in_maps, core_ids=list(range(N_CORES)))
    return assemble_output(res.results)


# revision 15
# speedup vs baseline: 1.0725x; 1.0115x over previous
"""Hyperbolic (Poincare-ball) average pooling 1D — Trainium2 Bass kernel (v2).

Problem: x (16, 256, 16384) f32, kernel=stride=4, manifold dim = channels (256).
Math (c=1), per window position:
    n2   = sum_C x^2                     (per input position)
    r    = 1/(1-n2)
    num  = sum_j r_j x_j  (window of 4)  ; den = sum_j r_j ; D = den - 2
    out  = num * g,  g = 1/(D + sqrt(D^2 - s)),  s = sum_C num^2

v2 strategy (all bf16 on the wire, f32 accumulation):
 - shard over batch (2 rows/core); host pre-transposes to (positions,
   channels+ones+pad) bf16 tiles [32, 128, 8*258].
 - n2: ScalarE/DVE square (bf16), then a PE identity-matmul 4:1 fold into
   PSUM ([128,8,64]) and a short DVE tensor_reduce — cuts the 1x reduce cost.
 - r = 1/(1-n2) via DVE iterative reciprocal (exact); no ACT ln/exp tables.
 - window sum as PE matmuls with 32-column stationary (mask32 * r) via
   col-tiling (tile_position), 258-wide moving incl. ones column for den.
 - evac num||den to bf16 SBUF (ScalarE), s via DVE TT-square (2x) + reduce,
   g chain on [128,16] batches: ScalarE Square/Sqrt + DVE reciprocal.
 - out = num * g via DVE tensor_scalar (2x), one 1MB DMA per 8 tiles.
"""

import sys

sys.path.insert(0, "/opt/trn_rl_repo")

import copy
import numpy as np
from ml_dtypes import bfloat16

import concourse.bass as bass
import concourse.mybir as mybir
from concourse import tile
from concourse.bass_utils import run_bass_kernel_spmd
from contextlib import ExitStack

F32 = mybir.dt.float32
BF16 = mybir.dt.bfloat16
AF = mybir.ActivationFunctionType
ALU = mybir.AluOpType

B, C, L = 16, 256, 16384
KERN = 4
T = L // KERN             # 4096 out positions per batch row
N_CORES = 8
B_PER = B // N_CORES      # 2
POS = B_PER * L           # 32768 input positions per core
OPOS = POS // KERN        # 8192 out positions per core
CPC = 258                 # 256 channels + ones col + zero pad
Q = 8                     # q-slots per x-tile
TILE_POS = 128 * Q        # 1024 input positions per x-tile
N_TILES = POS // TILE_POS  # 32
RG = 4                    # tiles per r-chain batch
GRP = 8                   # tiles per output group (g-chain, out DMA)
NG = N_TILES // GRP       # 4

# --- tuning knobs ---
N_SQ_DVE = 6      # how many tiles (of 32) square on DVE instead of ScalarE
WR_POOL = True    # build wr32 on GpSimd(Pool) instead of DVE
TTSQ_POOL = True  # square numsb on Pool instead of DVE


def _split_multi_waits(nc, max_waits=1):
    """walrus in this container rejects >1 sync-wait on one instruction;
    split extras into preceding single-wait NOPs on the same engine."""
    n_new = 0
    for bb in nc.m.functions[0].blocks:
        new_list = []
        for inst in bb.instructions:
            si = getattr(inst, "sync_info", None)
            if si is not None and si.on_wait and len(si.on_wait) > max_waits:
                extra = si.on_wait[:-max_waits]
                si_keep = si.on_wait[-max_waits:]
                for w in extra:
                    nop = mybir.InstNoOp(
                        name=f"{inst.name}-wsplit{n_new}", ins=[], outs=[]
                    )
                    nop.engine = inst.engine
                    nsi = copy.deepcopy(si)
                    nsi.on_wait = [w]
                    nsi.on_update = []
                    nop.sync_info = nsi
                    new_list.append(nop)
                    n_new += 1
                si.on_wait = si_keep
            new_list.append(inst)
        bb.instructions = new_list
    return n_new


def _register_const_ap(nc, value):
    t = nc.alloc_sbuf_tensor(f"const-float32-{value}", [128, 1], F32)
    nc.gpsimd.memset(t.ap(), value)
    nc.const_aps.aps[(F32, value)] = t.ap()


def build_nc(split_waits=True):
    nc = bass.Bass()
    _register_const_ap(nc, -2.0)
    nc.all_engine_barrier()
    xt = nc.declare_dram_parameter("xt", [N_TILES, 128, Q * CPC], BF16, isOutput=False)
    mask32 = nc.declare_dram_parameter("mask32", [128, 32], BF16, isOutput=False)
    mask8 = nc.declare_dram_parameter("mask8", [128, Q * 32], BF16, isOutput=False)
    masktm = nc.declare_dram_parameter("masktm", [128, 32 * Q], BF16, isOutput=False)
    ident = nc.declare_dram_parameter("ident", [128, 128], BF16, isOutput=False)
    out = nc.declare_dram_parameter("out", [NG, 128, GRP * 2 * 256], BF16, isOutput=True)

    NRG = N_TILES // RG  # 8 r-groups

    with tile.TileContext(nc) as tc:
        with ExitStack() as ctx:
            xpool = ctx.enter_context(tc.tile_pool(name="x", bufs=16))
            sqpool = ctx.enter_context(tc.tile_pool(name="sq", bufs=6))
            wrpool = ctx.enter_context(tc.tile_pool(name="wr", bufs=4))
            nbpool = ctx.enter_context(tc.tile_pool(name="nb", bufs=3))
            snpool = ctx.enter_context(tc.tile_pool(name="sn", bufs=3))
            stpool = ctx.enter_context(tc.tile_pool(name="st", bufs=3))
            opool = ctx.enter_context(tc.tile_pool(name="o", bufs=2))
            cpool = ctx.enter_context(tc.tile_pool(name="c", bufs=1))
            mpspool = ctx.enter_context(tc.tile_pool(name="mps", bufs=4, space="PSUM"))
            fpspool = ctx.enter_context(tc.tile_pool(name="fps", bufs=3, space="PSUM"))

            mask_t = cpool.tile([128, 32], BF16, tag="mask")
            nc.sync.dma_start(mask_t[:], mask32[:, :])
            mask8_t = cpool.tile([128, Q, 32], BF16, tag="mask8")
            nc.sync.dma_start(mask8_t[:], mask8.rearrange("p (q t) -> p q t", q=Q))
            masktm_t = cpool.tile([128, 32, Q], BF16, tag="masktm")
            nc.sync.dma_start(masktm_t[:], masktm.rearrange("p (t q) -> p t q", t=32))
            id_t = cpool.tile([128, 128], BF16, tag="id")
            nc.sync.dma_start(id_t[:], ident[:, :])

            # group state (created lazily per group)
            numsb_g = {}
            s_gs = {}
            o_ts = {}

            def group_state(grp):
                if grp not in numsb_g:
                    numsb_g[grp] = nbpool.tile([128, GRP, 2, CPC], BF16, tag="numsb", name=f"numsb{grp}")
                    s_gs[grp] = stpool.tile([128, 2 * GRP], F32, tag="s", name=f"sg{grp}")
                    o_ts[grp] = opool.tile([128, GRP, 2, 256], BF16, tag="o", name=f"og{grp}")
                return numsb_g[grp], s_gs[grp], o_ts[grp]

            a_state = {}   # tile i -> x_t
            m_state = {}   # rg -> m_g
            r_state = {}   # rg -> r_g

            def dma_tile(i):
                x_t = xpool.tile([128, Q, CPC], BF16, tag="x", name=f"x{i}")
                nc.sync.dma_start(x_t[:], xt[i].rearrange("p (q c) -> p q c", q=Q))
                a_state[i] = x_t

            sq_state = {}

            def a1_tile(i):
                """square for one tile."""
                x_t = a_state[i]
                sq_t = sqpool.tile([128, Q, 256], BF16, tag="sq")
                if N_SQ_DVE > 0 and i % (N_TILES // max(N_SQ_DVE, 1)) == 0:
                    nc.vector.tensor_tensor(
                        out=sq_t[:],
                        in0=x_t[:, :, 0:256],
                        in1=x_t[:, :, 0:256],
                        op=ALU.mult,
                    )
                else:
                    nc.scalar.activation(sq_t[:], x_t[:, :, 0:256], AF.Square)
                sq_state[i] = sq_t

            def a2_tile(i):
                """PE-fold + n2 reduce for one tile."""
                rg = i // RG
                if i % RG == 0:
                    m_state[rg] = stpool.tile(
                        [128, Q * RG], F32, tag="m", name=f"m{rg}"
                    )
                m_g = m_state[rg]
                j = i % RG
                sq_t = sq_state.pop(i)
                fps = fpspool.tile([128, Q, 64], F32, tag="fps")
                for ck in range(4):
                    nc.tensor.matmul(
                        fps[:],
                        id_t[:],
                        sq_t[:, :, 64 * ck : 64 * (ck + 1)],
                        start=(ck == 0),
                        stop=(ck == 3),
                    )
                nc.vector.tensor_reduce(
                    m_g[:, j * Q : (j + 1) * Q],
                    fps[:],
                    axis=mybir.AxisListType.X,
                    op=ALU.add,
                )

            def r_chain(rg):
                """r = 1/(1 - n2) for one batch of RG tiles."""
                m_g = m_state.pop(rg)
                w1 = stpool.tile([128, Q * RG], F32, tag="w1")
                nc.vector.tensor_scalar(
                    out=w1[:], in0=m_g[:], scalar1=-1.0, scalar2=1.0,
                    op0=ALU.mult, op1=ALU.add,
                )
                r_g = stpool.tile([128, Q * RG], F32, tag="r")
                nc.vector.reciprocal(r_g[:], w1[:])
                r_state[rg] = r_g

            wr_state = {}

            def b0_tile(i):
                """wr stationary build for one tile."""
                rg, j = i // RG, i % RG
                r_g = r_state[rg]
                wr_t = wrpool.tile([128, Q, 32], BF16, tag="wr")
                r_b = (
                    r_g[:, j * Q : (j + 1) * Q]
                    .rearrange("p (q o) -> p q o", o=1)
                    .broadcast_to([128, Q, 32])
                )
                eng = nc.gpsimd if WR_POOL else nc.vector
                eng.tensor_tensor(out=wr_t[:], in0=mask8_t[:], in1=r_b, op=ALU.mult)
                wr_state[i] = wr_t

            def b1_tile(i):
                """window matmuls + evac for one tile."""
                grp = i // GRP
                numsb, s_g, o_t = group_state(grp)
                x_t = a_state.pop(i)
                wr_t = wr_state.pop(i)
                jj = i % GRP
                pss = []
                for bk in range(2):
                    ps = mpspool.tile([128, CPC], F32, tag="ps")
                    for ql in range(4):
                        q = 4 * bk + ql
                        nc.tensor.matmul(
                            ps[32 * ql : 32 * (ql + 1), :],
                            wr_t[:, q],
                            x_t[:, q, :],
                            start=True,
                            stop=True,
                            tile_position=(0, 32 * ql),
                        )
                    pss.append(ps)
                for bk in range(2):
                    nc.scalar.copy(numsb[:, jj, bk], pss[bk][:])

            def b2_tile(i):
                """s = sum num^2 for one tile."""
                grp = i // GRP
                numsb, s_g, o_t = group_state(grp)
                jj = i % GRP
                sqn = snpool.tile([128, 2, 256], BF16, tag="sqn")
                teng = nc.gpsimd if TTSQ_POOL else nc.vector
                teng.tensor_tensor(
                    out=sqn[:],
                    in0=numsb[:, jj, :, 0:256],
                    in1=numsb[:, jj, :, 0:256],
                    op=ALU.mult,
                )
                nc.vector.tensor_reduce(
                    s_g[:, 2 * jj : 2 * jj + 2],
                    sqn[:],
                    axis=mybir.AxisListType.X,
                    op=ALU.add,
                )

            def phase_tail(grp):
                """den, g chain, out scale, store for one group of GRP tiles."""
                numsb = numsb_g.pop(grp)
                s_g = s_gs.pop(grp)
                o_t = o_ts.pop(grp)
                den_s = stpool.tile([128, 2 * GRP], F32, tag="den")
                nc.vector.tensor_copy(den_s[:], numsb[:, :, :, 256])
                d2 = stpool.tile([128, 2 * GRP], F32, tag="d2")
                nc.scalar.activation(d2[:], den_s[:], AF.Square, bias=-2.0)
                q2 = stpool.tile([128, 2 * GRP], F32, tag="q2")
                nc.vector.tensor_tensor(
                    out=q2[:], in0=d2[:], in1=s_g[:], op=ALU.subtract
                )
                u = stpool.tile([128, 2 * GRP], F32, tag="u")
                nc.scalar.activation(u[:], q2[:], AF.Sqrt)
                du = stpool.tile([128, 2 * GRP], F32, tag="du")
                nc.vector.scalar_tensor_tensor(
                    out=du[:], in0=den_s[:], scalar=-2.0, in1=u[:],
                    op0=ALU.add, op1=ALU.add,
                )
                g_s = stpool.tile([128, 2 * GRP], F32, tag="g")
                nc.vector.reciprocal(g_s[:], du[:])
                for jj in range(GRP):
                    for bk in range(2):
                        nc.vector.tensor_scalar(
                            out=o_t[:, jj, bk],
                            in0=numsb[:, jj, bk, 0:256],
                            scalar1=g_s[:, 2 * jj + bk : 2 * jj + bk + 1],
                            scalar2=None,
                            op0=ALU.mult,
                        )
                nc.scalar.dma_start(
                    out[grp], o_t[:].rearrange("p a b c -> p (a b c)")
                )

            # staged software pipeline (per-cycle emission order matters:
            # every engine's next instruction should have old, ready deps)
            LEAD = 7       # DMA leads a1
            LA2 = 2        # a2 (fold+TR) lags a1
            LB0 = 6        # b0 (wr) lag
            LB1 = 7        # b1 (MM+evac) lag
            LB2 = 9        # b2 (ttsq+sTR) lag
            TOT = LB2 + 1
            for i in range(min(LEAD, N_TILES)):
                dma_tile(i)
            for k in range(N_TILES + TOT):
                if k + LEAD < N_TILES:
                    dma_tile(k + LEAD)
                if k < N_TILES:
                    a1_tile(k)
                if RG <= k + RG - LB0 and k - LB0 < N_TILES and k >= LB0:
                    b0_tile(k - LB0)
                if k >= LB1 and k - LB1 < N_TILES:
                    b1_tile(k - LB1)
                if k >= LA2 and k - LA2 < N_TILES:
                    a2_tile(k - LA2)
                    if (k - LA2) % RG == RG - 1:
                        r_chain((k - LA2) // RG)
                if k >= LB2 and k - LB2 < N_TILES:
                    i2 = k - LB2
                    b2_tile(i2)
                    if i2 % GRP == GRP - 1:
                        phase_tail(i2 // GRP)

    if split_waits:
        _split_multi_waits(nc)
    return nc


_NC_CACHE = None


def _get_nc():
    global _NC_CACHE
    if _NC_CACHE is None:
        _NC_CACHE = build_nc()
    return _NC_CACHE


def _make_mask32():
    m = np.zeros((128, 32), dtype=np.float32)
    p = np.arange(128)
    m[p, p // 4] = 1.0
    return m.astype(bfloat16)


def prepare_core_inputs(x):
    """x: (16, 256, 16384) f32 -> list of per-core input dicts (bf16)."""
    mask32 = _make_mask32()
    mask8 = np.ascontiguousarray(np.tile(mask32.astype(np.float32), (1, Q))).astype(bfloat16)
    masktm = np.ascontiguousarray(
        np.repeat(mask32.astype(np.float32), Q, axis=1)
    ).astype(bfloat16)
    ident = np.eye(128, dtype=np.float32).astype(bfloat16)
    in_maps = []
    for k in range(N_CORES):
        xs = x[k * B_PER : (k + 1) * B_PER]  # (2, 256, L)
        xtf = np.empty((POS, CPC), dtype=np.float32)
        xtf[:, :C] = xs.transpose(0, 2, 1).reshape(POS, C)
        xtf[:, C] = 1.0
        xtf[:, C + 1] = 0.0
        # partition-major per-tile layout: (tile, p, q*CPC)
        xtb = np.ascontiguousarray(
            xtf.reshape(N_TILES, Q, 128, CPC).transpose(0, 2, 1, 3)
        ).reshape(N_TILES, 128, Q * CPC).astype(bfloat16)
        in_maps.append({"xt": xtb, "mask32": mask32, "mask8": mask8, "masktm": masktm, "ident": ident})
    return in_maps


def assemble_output(results):
    outs = []
    for k in range(N_CORES):
        o = results[k]["out"]  # (NG, 128, GRP*2*256) bf16
        o = np.asarray(o).astype(np.float32)
        # [NG, p=128, jj, bk, c] -> out position = ((g*GRP+jj)*2 + bk)*128 + p
        o = o.reshape(NG, 128, GRP, 2, 256).transpose(0, 2, 3, 1, 4).reshape(OPOS, 256)
        outs.append(o.reshape(B_PER, T, C).transpose(0, 2, 1))
    return np.ascontiguousarray(np.concatenate(outs, axis=0))


def kernel(x):
    x = np.ascontiguousarray(x, dtype=np.float32)
    nc = _get_nc()
    in_maps = prepare_core_inputs(x)
    res = run_bass_kernel_spmd(nc, in_maps, core_ids=list(range(N_CORES)))
    return assemble_output(res.results)
